# revision 1
# baseline (speedup 1.0000x reference)
"""Trainium2 Bass kernel for nn_LossConsistenciaMorfologicaCompuesta.

Composite morphological-consistency loss:
  for k in (3,5,7): Dice(pred, dilate_k(teacher)) + Dice(pred, erode_k(teacher)),
  total/3, where the structuring elements are cv2-style ellipses and Dice
  reduces over (batch, pixels).

Strategy (8 NeuronCores, data-parallel over batch B=16 -> 2 images/core):
  - Slab layout: one 1024x1024 image lives in SBUF as [128 partitions, 8+halo
    rows, 1024(+pad) cols] fp16. Vertical +-1/+-2 shifts become free-dim row
    offsets; the 2 halo rows at each slab edge are gathered with tiny
    partition-shifted SBUF->SBUF DMAs. Out-of-image halo rows use replicate
    padding, which is exact for flat morphology (a duplicated in-window pixel
    never changes a max/min).
  - Ellipse decomposition (verified exact vs the reference):
      X1   = hmax3(t)
      dil3 = max(X1, t up1, t dn1)                      (ellipse 3 = plus)
      dil5 = max(dil3 l1, dil3 r1, dil3 up1, dil3 dn1)  (ellipse 5 = diamond2)
      dil7 = max(dil5 l1/r1/up1/dn1, (t+-2,+-2) corners) (ellipse 7)
    erosion mirrored with min.
  - Per-image sums: plain sums (sum m, sum p) ride the ScalarE activation
    accumulator; product sums (sum p*m) go through PE ones-matmuls into PSUM.
  - Each core writes 22 partial sums; the host combines them into the scalar.
"""

import numpy as np

B, C_IN, H, W = 16, 1, 1024, 1024
NCORES = 8
BPC = B // NCORES      # images per core
P = 128                # SBUF partitions
R = H // P             # 8 slab rows per partition
EPS = 1e-7
PSUM_CHUNK = 512

_CACHE = {}


def build_nc(n_img=BPC, rows=R, cols=W):
    """Emit the Bass program for one core processing n_img images of
    (rows*128) x cols."""
    import concourse.bacc as bacc
    import concourse.mybir as mybir
    import concourse.tile as tile

    f32 = mybir.dt.float32
    f16 = mybir.dt.float16
    MAX = mybir.AluOpType.max
    MIN = mybir.AluOpType.min
    MULT = mybir.AluOpType.mult
    COPY = mybir.ActivationFunctionType.Copy

    Rr, C = rows, cols
    TROWS = Rr + 4          # t: 2 halo rows above + below
    MROWS = Rr + 2          # m3/m5 buffers: 1 halo row above + below
    MC = C + 4              # 2 pad cols each side
    WPLAIN = 16             # plain-sum accumulator columns
    NQ = 6                  # morph quantities: d3,d5,d7,e3,e5,e7

    nc = bacc.Bacc("TRN2", target_bir_lowering=False)
    t_dram = nc.dram_tensor("teacher", [n_img, Rr * P, C], f32, kind="ExternalInput")
    p_dram = nc.dram_tensor("pred", [n_img, Rr * P, C], f32, kind="ExternalInput")
    out_dram = nc.dram_tensor("partials", [1, 6 + WPLAIN], f32, kind="ExternalOutput")

    def halo(m):
        """Fill 1-row top/bottom halos of a morph buffer (replicate at image
        edges); pad columns ride along."""
        nc.sync.dma_start(m[1:P, 0:1, :], m[0:P - 1, MROWS - 2:MROWS - 1, :])
        nc.sync.dma_start(m[0:P - 1, MROWS - 1:MROWS, :], m[1:P, 1:2, :])
        nc.sync.dma_start(m[0:1, 0:1, :], m[0:1, 1:2, :])
        nc.sync.dma_start(m[P - 1:P, MROWS - 1:MROWS, :],
                          m[P - 1:P, MROWS - 2:MROWS - 1, :])

    with tile.TileContext(nc) as tc:
        with (
            tc.tile_pool(name="stage", bufs=2) as stage_pool,
            tc.tile_pool(name="img", bufs=1) as img_pool,
            tc.tile_pool(name="morph", bufs=1) as morph_pool,
            tc.tile_pool(name="m7", bufs=2) as m7_pool,
            tc.tile_pool(name="small", bufs=1) as small_pool,
            tc.tile_pool(name="psum", bufs=1, space="PSUM") as psum_pool,
        ):
            sums = small_pool.tile([P, WPLAIN], f32, tag="sums")
            ones16 = small_pool.tile([P, 1], f16, tag="ones16")
            ones32 = small_pool.tile([P, 1], f32, tag="ones32")
            nc.vector.memset(sums[:], 0.0)
            nc.vector.memset(ones16[:], 1.0)
            nc.vector.memset(ones32[:], 1.0)

            # long-lived image buffers (reused across images/sides)
            t = img_pool.tile([P, TROWS, C], f16, tag="t")
            p = img_pool.tile([P, Rr, C], f16, tag="p")
            h1 = morph_pool.tile([P, Rr, C], f16, tag="h1")
            mbuf = {}
            for pref, fill in (("d", -1e4), ("e", 1e4)):
                for lvl in ("3", "5"):
                    m = morph_pool.tile([P, MROWS, MC], f16, tag=pref + lvl, name=pref + lvl)
                    nc.vector.memset(m[:, :, 0:2], fill)
                    nc.vector.memset(m[:, :, MC - 2:MC], fill)
                    mbuf[pref + lvl] = m

            ps_prod = [psum_pool.tile([1, min(PSUM_CHUNK, C)], f32, tag=f"ps{q}", name=f"ps{q}")
                       for q in range(NQ)]
            n_chunks = Rr * ((C + PSUM_CHUNK - 1) // PSUM_CHUNK)
            total_mm = n_img * n_chunks
            mm_count = [0] * NQ

            def pe_sum(q, m_ap):
                """Accumulate sum over a [P, Rr, C] AP into ps_prod[q]."""
                for r in range(Rr):
                    for c0 in range(0, C, PSUM_CHUNK):
                        cw = min(PSUM_CHUNK, C - c0)
                        nc.tensor.matmul(
                            ps_prod[q][:, 0:cw],
                            ones16[:],
                            m_ap[:, r, c0:c0 + cw],
                            start=(mm_count[q] == 0),
                            stop=(mm_count[q] == total_mm - 1),
                        )
                        mm_count[q] += 1

            for img in range(n_img):
                # ---- load + cast to fp16 ----
                t_view = t_dram[img].rearrange("(p r) w -> p r w", p=P)
                p_view = p_dram[img].rearrange("(p r) w -> p r w", p=P)
                CH = 2  # slab rows per staging chunk
                for r0 in range(0, Rr, CH):
                    st = stage_pool.tile([P, CH, C], f32, tag="stage", name="stage")
                    nc.sync.dma_start(st[:], t_view[:, r0:r0 + CH, :])
                    nc.scalar.activation(t[:, 2 + r0:2 + r0 + CH, :], st[:], COPY)
                for r0 in range(0, Rr, CH):
                    st = stage_pool.tile([P, CH, C], f32, tag="stage", name="stage")
                    nc.sync.dma_start(st[:], p_view[:, r0:r0 + CH, :])
                    nc.scalar.activation(p[:, r0:r0 + CH, :], st[:], COPY)

                # ---- t halo rows (2 each side, replicate at image boundary) ----
                nc.sync.dma_start(t[1:P, 0:2, :], t[0:P - 1, Rr:Rr + 2, :])
                nc.sync.dma_start(t[0:P - 1, TROWS - 2:TROWS, :], t[1:P, 2:4, :])
                for hr in (0, 1):
                    nc.sync.dma_start(t[0:1, hr:hr + 1, :], t[0:1, 2:3, :])
                for hr in (TROWS - 2, TROWS - 1):
                    nc.sync.dma_start(t[P - 1:P, hr:hr + 1, :],
                                      t[P - 1:P, TROWS - 3:TROWS - 2, :])

                # sum(p) per partition on ACT (in-place identity copy)
                nc.scalar.activation(p[:], p[:], COPY,
                                     accum_out=sums[:, img:img + 1])

                for is_dil, base_q, pref in ((True, 0, "d"), (False, 3, "e")):
                    OP = MAX if is_dil else MIN

                    # ---- h1 = hmax3/hmin3 of t ----
                    nc.vector.tensor_tensor(h1[:, :, 1:C - 1], t[:, 2:2 + Rr, 0:C - 2],
                                            t[:, 2:2 + Rr, 2:C], op=OP)
                    nc.vector.tensor_tensor(h1[:, :, 1:C - 1], h1[:, :, 1:C - 1],
                                            t[:, 2:2 + Rr, 1:C - 1], op=OP)
                    nc.vector.tensor_tensor(h1[:, :, 0:1], t[:, 2:2 + Rr, 0:1],
                                            t[:, 2:2 + Rr, 1:2], op=OP)
                    nc.vector.tensor_tensor(h1[:, :, C - 1:C], t[:, 2:2 + Rr, C - 2:C - 1],
                                            t[:, 2:2 + Rr, C - 1:C], op=OP)

                    # ---- m3 = op(h1, t up1, t dn1) ----
                    m3 = mbuf[pref + "3"]
                    nc.vector.tensor_tensor(m3[:, 1:1 + Rr, 2:C + 2], h1[:, :, :],
                                            t[:, 3:3 + Rr, :], op=OP)
                    nc.vector.tensor_tensor(m3[:, 1:1 + Rr, 2:C + 2],
                                            m3[:, 1:1 + Rr, 2:C + 2],
                                            t[:, 1:1 + Rr, :], op=OP)
                    halo(m3)

                    # ---- m5 = op(m3 l1, r1, up1, dn1) ----
                    m5 = mbuf[pref + "5"]
                    nc.vector.tensor_tensor(m5[:, 1:1 + Rr, 2:C + 2],
                                            m3[:, 1:1 + Rr, 1:C + 1],
                                            m3[:, 1:1 + Rr, 3:C + 3], op=OP)
                    nc.vector.tensor_tensor(m5[:, 1:1 + Rr, 2:C + 2],
                                            m5[:, 1:1 + Rr, 2:C + 2],
                                            m3[:, 2:2 + Rr, 2:C + 2], op=OP)
                    nc.vector.tensor_tensor(m5[:, 1:1 + Rr, 2:C + 2],
                                            m5[:, 1:1 + Rr, 2:C + 2],
                                            m3[:, 0:Rr, 2:C + 2], op=OP)
                    halo(m5)

                    # ---- m7 = op(m5 l1/r1/up1/dn1, t corner terms) ----
                    m7 = m7_pool.tile([P, Rr, C], f16, tag="m7", name="m7")
                    nc.vector.tensor_tensor(m7[:], m5[:, 1:1 + Rr, 1:C + 1],
                                            m5[:, 1:1 + Rr, 3:C + 3], op=OP)
                    nc.vector.tensor_tensor(m7[:], m7[:],
                                            m5[:, 2:2 + Rr, 2:C + 2], op=OP)
                    nc.vector.tensor_tensor(m7[:], m7[:],
                                            m5[:, 0:Rr, 2:C + 2], op=OP)
                    # corners: (t up2 / dn2) shifted +-2 cols, col-restricted
                    nc.vector.tensor_tensor(m7[:, :, 2:C], m7[:, :, 2:C],
                                            t[:, 4:4 + Rr, 0:C - 2], op=OP)
                    nc.vector.tensor_tensor(m7[:, :, 0:C - 2], m7[:, :, 0:C - 2],
                                            t[:, 4:4 + Rr, 2:C], op=OP)
                    nc.vector.tensor_tensor(m7[:, :, 2:C], m7[:, :, 2:C],
                                            t[:, 0:Rr, 0:C - 2], op=OP)
                    nc.vector.tensor_tensor(m7[:, :, 0:C - 2], m7[:, :, 0:C - 2],
                                            t[:, 0:Rr, 2:C], op=OP)

                    # ---- sums + products ----
                    col = 2 + img * 6
                    m3i = m3[:, 1:1 + Rr, 2:C + 2]
                    m5i = m5[:, 1:1 + Rr, 2:C + 2]
                    for qi, m_ap in ((0, m3i), (1, m5i), (2, m7[:, :, :])):
                        q = base_q + qi
                        nc.scalar.activation(m_ap, m_ap, COPY,
                                             accum_out=sums[:, col + q:col + q + 1])
                        nc.vector.tensor_tensor(m_ap, m_ap, p[:], op=MULT)
                        pe_sum(q, m_ap)

            # ---- epilogue ----
            CW = min(PSUM_CHUNK, C)
            prodsb = small_pool.tile([1, NQ * CW], f32, tag="prodsb")
            outsb = small_pool.tile([1, 6 + WPLAIN], f32, tag="outsb")
            for q in range(NQ):
                nc.scalar.activation(prodsb[:, q * CW:(q + 1) * CW],
                                     ps_prod[q][:], COPY)
            nc.vector.tensor_reduce(
                outsb[:, 0:NQ],
                prodsb[:, :].rearrange("p (q k) -> p q k", k=CW),
                axis=mybir.AxisListType.X,
                op=mybir.AluOpType.add,
            )
            ps_plain = psum_pool.tile([1, WPLAIN], f32, tag="psplain")
            nc.tensor.matmul(ps_plain[:], ones32[:], sums[:], start=True, stop=True)
            nc.scalar.activation(outsb[:, NQ:NQ + WPLAIN], ps_plain[:], COPY)
            nc.sync.dma_start(out_dram[:], outsb[:])

    nc.compile()
    return nc


def combine_partials(partials, n_img=BPC):
    """partials: [ncores, 22] float32 -> scalar loss (mirrors reference math)."""
    partials = np.asarray(partials, dtype=np.float64)
    prod_sums = partials[:, 0:6].sum(axis=0)            # sum p*m per quantity
    plain = partials[:, 6:]                             # [ncores, 16]
    p_sum = plain[:, 0:n_img].sum()
    m_sums = np.zeros(6)
    for img in range(n_img):
        m_sums += plain[:, 2 + img * 6:2 + img * 6 + 6].sum(axis=0)
    total = 0.0
    for q in range(6):
        card = p_sum + m_sums[q]
        score = 2.0 * prod_sums[q] / max(card, EPS)
        loss = (1.0 - score) * (1.0 if m_sums[q] > 0 else 0.0)
        total += loss
    return np.float32(total / 3.0)


def kernel(pred_student_prob, teacher_prob):
    from concourse.bass_utils import run_bass_kernel_spmd

    key = (BPC, R, W)
    if key not in _CACHE:
        _CACHE[key] = build_nc(BPC, R, W)
    nc = _CACHE[key]

    pred = np.ascontiguousarray(pred_student_prob.reshape(B, H, W), dtype=np.float32)
    teach = np.ascontiguousarray(teacher_prob.reshape(B, H, W), dtype=np.float32)
    in_maps = []
    for c in range(NCORES):
        sl = slice(c * BPC, (c + 1) * BPC)
        in_maps.append({
            "teacher": np.ascontiguousarray(teach[sl]),
            "pred": np.ascontiguousarray(pred[sl]),
        })
    res = run_bass_kernel_spmd(nc, in_maps, core_ids=list(range(NCORES)))
    partials = np.stack([res.results[c]["partials"][0] for c in range(NCORES)])
    return combine_partials(partials)



# revision 7
# speedup vs baseline: 1.3380x; 1.3380x over previous
"""Trainium2 Bass kernel for nn_LossConsistenciaMorfologicaCompuesta.

Composite morphological-consistency loss:
  for k in (3,5,7): Dice(pred, dilate_k(teacher)) + Dice(pred, erode_k(teacher)),
  total/3, cv2-ellipse structuring elements, Dice reduced over (batch, pixels).

Strategy (8 NeuronCores, data-parallel over batch B=16 -> 2 images/core):
  - Slab layout: one 1024x1024 image in SBUF as [128 partitions, 8 rows(+halo),
    1024 cols] fp16 (DVE 2x mode). Vertical shifts are free-dim row offsets;
    halo rows come from tiny partition-shifted SBUF->SBUF DMAs (replicate at
    image edges -- exact for flat morphology).
  - Ellipse decomposition (exact, 13 DVE folds per side):
      h1   = hmax3(t);  m3 = max(h1, t up1, t dn1)          (ellipse3 = plus)
      m5   = max(m3 l1, r1, up1, dn1)                        (ellipse5 = diamond2)
      hm2  = max(t l2, t r2)  (halo rows via DMA)
      m7   = max(hm2 up2, hm2 dn2, m5 l1, r1, up1, dn1)     (ellipse7)
    erosion mirrored with min. DVE is the only engine with elementwise
    max/min on this target, so everything else rides other engines:
  - sum(p*m) on PE: "diagonal" matmuls (weights = 128-col blocks of p,
    moving = m blocks) accumulated into a [128,128] PSUM region whose
    diagonal holds the inter partials; extracted once at the end with a
    tensor_tensor_reduce against an identity matrix.
  - sum(m) on PE: ones-weight matmuls into a [1,384] PSUM region sharing
    the same bank as the diag tile (PSUM is bank-granular, 8 banks).
  - sum(p) rides the f32->f16 cast on ACT as accum_out.
  - Each core writes 20 partial sums; the host combines into the scalar.
"""

import numpy as np

B, C_IN, H, W = 16, 1, 1024, 1024
NCORES = 8
BPC = B // NCORES      # images per core
P = 128                # SBUF partitions
R = H // P             # 8 slab rows per partition
EPS = 1e-7
CH = 2                 # slab rows per f32 staging chunk
NSUM = 26              # sums cols: 8 p-cast + 6 inter + 12 msum
NOUT = 26              # output partials per core

_CACHE = {}


def build_nc():
    import concourse.bacc as bacc
    import concourse.mybir as mybir
    import concourse.tile as tile

    f32 = mybir.dt.float32
    f16 = mybir.dt.float16
    MAX = mybir.AluOpType.max
    MIN = mybir.AluOpType.min
    ADD = mybir.AluOpType.add
    MULT = mybir.AluOpType.mult
    COPY = mybir.ActivationFunctionType.Copy

    C = W
    n_img = BPC
    TROWS = R + 4   # t/hm2: 2 halo rows above+below
    MROWS = R + 2   # m3/m5: 1 halo row above+below
    MC = C + 2      # m3/m5: 1 pad col each side (buf col = img col + 1)

    nc = bacc.Bacc("TRN2", target_bir_lowering=False)
    t_dram = nc.dram_tensor("teacher", [n_img, R * P, C], f32, kind="ExternalInput")
    p_dram = nc.dram_tensor("pred", [n_img, R * P, C], f32, kind="ExternalInput")
    i_dram = nc.dram_tensor("ident", [P, P], f32, kind="ExternalInput")
    out_dram = nc.dram_tensor("partials", [1, NOUT], f32, kind="ExternalOutput")

    with tile.TileContext(nc) as tc:
        with (
            tc.tile_pool(name="stage", bufs=2) as stage_pool,
            tc.tile_pool(name="img", bufs=2) as img_pool,
            tc.tile_pool(name="morph", bufs=1) as morph_pool,
            tc.tile_pool(name="small", bufs=1) as small_pool,
            tc.tile_pool(name="psum", bufs=1, space="PSUM") as psum_pool,
        ):
            sums = small_pool.tile([P, NSUM], f32, tag="sums")
            ones16 = small_pool.tile([P, 1], f16, tag="ones16")
            ones32 = small_pool.tile([P, 1], f32, tag="ones32")
            ident = small_pool.tile([P, P], f32, tag="ident")
            scr = small_pool.tile([P, P], f32, tag="scr")
            outsb = small_pool.tile([1, NOUT], f32, tag="outsb")
            nc.vector.memset(sums[:], 0.0)
            nc.vector.memset(ones16[:], 1.0)
            nc.vector.memset(ones32[:], 1.0)
            nc.sync.dma_start(ident[:], i_dram[:])

            h1 = morph_pool.tile([P, R, C], f16, tag="h1")
            m3 = morph_pool.tile([P, MROWS, MC], f16, tag="m3")
            m5 = morph_pool.tile([P, MROWS, MC], f16, tag="m5")
            hm2 = morph_pool.tile([P, TROWS, C], f16, tag="hm2")
            m7 = morph_pool.tile([P, R, C], f16, tag="m7")

            # one PSUM bank per quantity holding the [128,128] diag tile
            ps_q = [psum_pool.tile([P, P], f32, tag=f"psq{q}", name=f"psq{q}")
                    for q in range(6)]
            diag_cnt = [0] * 6
            iblk = [(c0, 128) for c0 in range(0, C, 128)]
            DIAG_TOT = n_img * R * len(iblk)

            def halo1(m):
                """1-row top/bottom halo fill (replicate at image edges)."""
                nc.sync.dma_start(m[1:P, 0:1, :], m[0:P - 1, R:R + 1, :])
                nc.sync.dma_start(m[0:P - 1, MROWS - 1:MROWS, :], m[1:P, 1:2, :])
                nc.sync.dma_start(m[0:1, 0:1, :], m[0:1, 1:2, :])
                nc.sync.dma_start(m[P - 1:P, MROWS - 1:MROWS, :],
                                  m[P - 1:P, MROWS - 2:MROWS - 1, :])

            def halo2(m):
                """2-row top/bottom halo fill for a TROWS buffer."""
                nc.sync.dma_start(m[1:P, 0:2, :], m[0:P - 1, R:R + 2, :])
                nc.sync.dma_start(m[0:P - 1, TROWS - 2:TROWS, :], m[1:P, 2:4, :])
                for hr in (0, 1):
                    nc.sync.dma_start(m[0:1, hr:hr + 1, :], m[0:1, 2:3, :])
                for hr in (TROWS - 2, TROWS - 1):
                    nc.sync.dma_start(m[P - 1:P, hr:hr + 1, :],
                                      m[P - 1:P, TROWS - 3:TROWS - 2, :])

            def pe_sums(q, img, p_t, buf, rofs, cofs):
                """sum(m) rides ACT (in-place copy + accum); sum(p*m) on PE
                via diagonal-accumulate matmuls."""
                m_ap = buf[:, rofs:rofs + R, cofs:cofs + C]
                nc.scalar.activation(m_ap, m_ap, COPY,
                                     accum_out=sums[:, 14 + img * 6 + q:
                                                    15 + img * 6 + q])
                for r in range(R):
                    for (c0, cw) in iblk:
                        nc.tensor.matmul(
                            ps_q[q][0:cw, 0:cw], p_t[:, r, c0:c0 + cw],
                            buf[:, rofs + r, cofs + c0:cofs + c0 + cw],
                            start=(diag_cnt[q] == 0),
                            stop=(diag_cnt[q] == DIAG_TOT - 1))
                        diag_cnt[q] += 1

            for img in range(n_img):
                t_view = t_dram[img].rearrange("(p r) w -> p r w", p=P)
                p_view = p_dram[img].rearrange("(p r) w -> p r w", p=P)
                t = img_pool.tile([P, TROWS, C], f16, tag="t", name="t")
                p_t = img_pool.tile([P, R, C], f16, tag="p", name="p")
                for r0 in range(0, R, CH):
                    st = stage_pool.tile([P, CH, C], f32, tag="stage", name="stage")
                    nc.sync.dma_start(st[:], t_view[:, r0:r0 + CH, :])
                    nc.scalar.activation(t[:, 2 + r0:2 + r0 + CH, :], st[:], COPY)
                for r0 in range(0, R, CH):
                    st = stage_pool.tile([P, CH, C], f32, tag="stage", name="stage")
                    nc.sync.dma_start(st[:], p_view[:, r0:r0 + CH, :])
                    nc.scalar.activation(p_t[:, r0:r0 + CH, :], st[:], COPY,
                                         accum_out=sums[:, img * 4 + r0 // CH:
                                                        img * 4 + r0 // CH + 1])
                halo2(t)

                for is_dil, base_q in ((True, 0), (False, 3)):
                    OP = MAX if is_dil else MIN
                    fill = -1e4 if is_dil else 1e4
                    V = nc.vector

                    V.memset(m3[:, :, 0:1], fill)
                    V.memset(m3[:, :, MC - 1:MC], fill)
                    V.memset(m5[:, :, 0:1], fill)
                    V.memset(m5[:, :, MC - 1:MC], fill)

                    # ---- h1 = hmax3(t) ----
                    V.tensor_tensor(h1[:, :, 1:C - 1], t[:, 2:10, 0:C - 2],
                                    t[:, 2:10, 2:C], op=OP)
                    V.tensor_tensor(h1[:, :, 0:1], t[:, 2:10, 1:2],
                                    t[:, 2:10, 1:2], op=OP)
                    V.tensor_tensor(h1[:, :, C - 1:C], t[:, 2:10, C - 2:C - 1],
                                    t[:, 2:10, C - 2:C - 1], op=OP)
                    V.tensor_tensor(h1[:], h1[:], t[:, 2:10, :], op=OP)

                    # ---- hm2 = op(t l2, t r2), halo rows via DMA ----
                    V.tensor_tensor(hm2[:, 2:10, 2:C - 2], t[:, 2:10, 0:C - 4],
                                    t[:, 2:10, 4:C], op=OP)
                    V.tensor_tensor(hm2[:, 2:10, 0:2], t[:, 2:10, 2:4],
                                    t[:, 2:10, 2:4], op=OP)
                    V.tensor_tensor(hm2[:, 2:10, C - 2:C], t[:, 2:10, C - 4:C - 2],
                                    t[:, 2:10, C - 4:C - 2], op=OP)
                    halo2(hm2)

                    # ---- m3 = op(h1, t up1, t dn1) ----
                    V.tensor_tensor(m3[:, 1:9, 1:C + 1], h1[:], t[:, 1:9, :], op=OP)
                    V.tensor_tensor(m3[:, 1:9, 1:C + 1], m3[:, 1:9, 1:C + 1],
                                    t[:, 3:11, :], op=OP)
                    halo1(m3)
                    pe_sums(base_q + 0, img, p_t, m3, 1, 1)

                    # ---- m5 = op(m3 l1, r1, up1, dn1) ----
                    V.tensor_tensor(m5[:, 1:9, 1:C + 1], m3[:, 1:9, 0:C],
                                    m3[:, 1:9, 2:C + 2], op=OP)
                    V.tensor_tensor(m5[:, 1:9, 1:C + 1], m5[:, 1:9, 1:C + 1],
                                    m3[:, 0:8, 1:C + 1], op=OP)
                    V.tensor_tensor(m5[:, 1:9, 1:C + 1], m5[:, 1:9, 1:C + 1],
                                    m3[:, 2:10, 1:C + 1], op=OP)
                    halo1(m5)
                    pe_sums(base_q + 1, img, p_t, m5, 1, 1)

                    # ---- m7 = op(hm2 up2, dn2, m5 l1, r1, up1, dn1) ----
                    V.tensor_tensor(m7[:], hm2[:, 0:8, :], hm2[:, 4:12, :], op=OP)
                    V.tensor_tensor(m7[:], m7[:], m5[:, 1:9, 0:C], op=OP)
                    V.tensor_tensor(m7[:], m7[:], m5[:, 1:9, 2:C + 2], op=OP)
                    V.tensor_tensor(m7[:], m7[:], m5[:, 0:8, 1:C + 1], op=OP)
                    V.tensor_tensor(m7[:], m7[:], m5[:, 2:10, 1:C + 1], op=OP)
                    pe_sums(base_q + 2, img, p_t, m7, 0, 0)

            # ---- epilogue ----
            scr2 = small_pool.tile([P, P], f32, tag="scr2")
            for q in range(6):
                nc.scalar.activation(scr[:], ps_q[q][:], COPY)
                nc.vector.tensor_tensor(scr2[:], scr[:], ident[:], op=MULT)
                nc.vector.tensor_reduce(sums[:, 8 + q:9 + q], scr2[:],
                                        axis=mybir.AxisListType.X, op=ADD)
            ps_fin = psum_pool.tile([1, NSUM], f32, tag="psfin")
            nc.tensor.matmul(ps_fin[:], ones32[:], sums[:], start=True, stop=True)
            nc.scalar.activation(outsb[:, 0:NSUM], ps_fin[:], COPY)
            nc.sync.dma_start(out_dram[:], outsb[:])

    nc.compile()
    return nc


def combine_partials(partials):
    """partials: [ncores, 26] float32 -> scalar loss (mirrors reference)."""
    partials = np.asarray(partials, dtype=np.float64)
    p_sum = partials[:, 0:8].sum()
    total = 0.0
    for q in range(6):
        inter = partials[:, 8 + q].sum()
        msum = sum(partials[:, 14 + img * 6 + q].sum() for img in range(BPC))
        card = p_sum + msum
        score = 2.0 * inter / max(card, EPS)
        total += (1.0 - score) * (1.0 if msum > 0 else 0.0)
    return np.float32(total / 3.0)


def kernel(pred_student_prob, teacher_prob):
    from concourse.bass_utils import run_bass_kernel_spmd

    if "nc" not in _CACHE:
        _CACHE["nc"] = build_nc()
    nc = _CACHE["nc"]

    pred = np.ascontiguousarray(pred_student_prob.reshape(B, H, W), dtype=np.float32)
    teach = np.ascontiguousarray(teacher_prob.reshape(B, H, W), dtype=np.float32)
    ident = np.eye(P, dtype=np.float32)
    in_maps = []
    for c in range(NCORES):
        sl = slice(c * BPC, (c + 1) * BPC)
        in_maps.append({
            "teacher": np.ascontiguousarray(teach[sl]),
            "pred": np.ascontiguousarray(pred[sl]),
            "ident": ident,
        })
    res = run_bass_kernel_spmd(nc, in_maps, core_ids=list(range(NCORES)))
    partials = np.stack([res.results[c]["partials"][0] for c in range(NCORES)])
    return combine_partials(partials)


# revision 11
# speedup vs baseline: 1.4196x; 1.0610x over previous
"""Trainium2 Bass kernel for nn_LossConsistenciaMorfologicaCompuesta.

Composite morphological-consistency loss:
  for k in (3,5,7): Dice(pred, dilate_k(teacher)) + Dice(pred, erode_k(teacher)),
  total/3, cv2-ellipse structuring elements, Dice reduced over (batch, pixels).

Strategy (8 NeuronCores, data-parallel over batch B=16 -> 2 images/core):
  - Slab layout: one 1024x1024 image in SBUF as [128 partitions, 8 rows(+halo),
    1024 cols] fp16 (DVE 2x mode). Vertical shifts are free-dim row offsets;
    halo rows come from tiny partition-shifted SBUF->SBUF DMAs (replicate at
    image edges -- exact for flat morphology).
  - Ellipse decomposition (exact, 13 DVE folds per side):
      h1   = hmax3(t);  m3 = max(h1, t up1, t dn1)          (ellipse3 = plus)
      m5   = max(m3 l1, r1, up1, dn1)                        (ellipse5 = diamond2)
      hm2  = max(t l2, t r2)  (halo rows via DMA)
      m7   = max(hm2 up2, hm2 dn2, m5 l1, r1, up1, dn1)     (ellipse7)
    erosion mirrored with min. DVE is the only engine with elementwise
    max/min on this target, so everything else rides other engines:
  - sum(p*m) on PE: "diagonal" matmuls (weights = 128-col blocks of p,
    moving = m blocks) accumulated into a [128,128] PSUM region whose
    diagonal holds the inter partials; extracted once at the end with a
    tensor_tensor_reduce against an identity matrix.
  - sum(m) on PE: ones-weight matmuls into a [1,384] PSUM region sharing
    the same bank as the diag tile (PSUM is bank-granular, 8 banks).
  - sum(p) rides the f32->f16 cast on ACT as accum_out.
  - Each core writes 20 partial sums; the host combines into the scalar.
"""

import numpy as np

B, C_IN, H, W = 16, 1, 1024, 1024
NCORES = 8
BPC = B // NCORES      # images per core
P = 128                # SBUF partitions
R = H // P             # 8 slab rows per partition
EPS = 1e-7
CH = 2                 # slab rows per f32 staging chunk
NSUM = 14              # sums cols: 8 p-cast + 6 inter
NOUT = 26              # 14 sums + 12 msum scalars

_CACHE = {}


def build_nc():
    import concourse.bacc as bacc
    import concourse.mybir as mybir
    import concourse.tile as tile

    f32 = mybir.dt.float32
    f16 = mybir.dt.float16
    MAX = mybir.AluOpType.max
    MIN = mybir.AluOpType.min
    ADD = mybir.AluOpType.add
    MULT = mybir.AluOpType.mult
    COPY = mybir.ActivationFunctionType.Copy

    C = W
    n_img = BPC
    TROWS = R + 4   # t/hm2: 2 halo rows above+below
    MROWS = R + 2   # m3/m5: 1 halo row above+below
    MC = C + 2      # m3/m5: 1 pad col each side (buf col = img col + 1)

    nc = bacc.Bacc("TRN2", target_bir_lowering=False)
    t_dram = nc.dram_tensor("teacher", [n_img, R * P, C], f32, kind="ExternalInput")
    p_dram = nc.dram_tensor("pred", [n_img, R * P, C], f32, kind="ExternalInput")
    i_dram = nc.dram_tensor("ident", [P, P], f32, kind="ExternalInput")
    out_dram = nc.dram_tensor("partials", [1, NOUT], f32, kind="ExternalOutput")

    with tile.TileContext(nc) as tc:
        with (
            tc.tile_pool(name="stage", bufs=2) as stage_pool,
            tc.tile_pool(name="img", bufs=2) as img_pool,
            tc.tile_pool(name="morph", bufs=1) as morph_pool,
            tc.tile_pool(name="small", bufs=1) as small_pool,
            tc.tile_pool(name="psum", bufs=1, space="PSUM") as psum_pool,
        ):
            sums = small_pool.tile([P, NSUM], f32, tag="sums")
            ones16 = small_pool.tile([P, 1], f16, tag="ones16")
            ones32 = small_pool.tile([P, 1], f32, tag="ones32")
            ident = small_pool.tile([P, P], f32, tag="ident")
            scr = small_pool.tile([P, P], f32, tag="scr")
            scr512 = small_pool.tile([1, 512], f32, tag="scr512")
            scr2 = small_pool.tile([P, P], f32, tag="scr2")
            msums1 = small_pool.tile([1, 12], f32, tag="msums1")
            outsb = small_pool.tile([1, NOUT], f32, tag="outsb")
            nc.vector.memset(sums[:], 0.0)
            nc.vector.memset(ones16[:], 1.0)
            nc.vector.memset(ones32[:], 1.0)
            nc.sync.dma_start(ident[:], i_dram[:])

            h1 = morph_pool.tile([P, R, C], f16, tag="h1")
            m3 = morph_pool.tile([P, MROWS, MC], f16, tag="m3")
            m5 = morph_pool.tile([P, MROWS, MC], f16, tag="m5")
            hm2 = morph_pool.tile([P, TROWS, C], f16, tag="hm2")
            m7 = morph_pool.tile([P, R, C], f16, tag="m7")

            # one PSUM bank per quantity holding the [128,128] diag tile,
            # plus a single shared [1,512] bank for the msum chains
            ps_q = [psum_pool.tile([P, P], f32, tag=f"psq{q}", name=f"psq{q}")
                    for q in range(6)]
            ps_ms = psum_pool.tile([1, 512], f32, tag="psms")
            diag_cnt = [0] * 6
            iblk = [(c0, 128) for c0 in range(0, C, 128)]
            DIAG_TOT = n_img * R * len(iblk)

            def halo1(m):
                """1-row top/bottom halo fill (replicate at image edges)."""
                nc.sync.dma_start(m[1:P, 0:1, :], m[0:P - 1, R:R + 1, :])
                nc.sync.dma_start(m[0:P - 1, MROWS - 1:MROWS, :], m[1:P, 1:2, :])
                nc.sync.dma_start(m[0:1, 0:1, :], m[0:1, 1:2, :])
                nc.sync.dma_start(m[P - 1:P, MROWS - 1:MROWS, :],
                                  m[P - 1:P, MROWS - 2:MROWS - 1, :])

            def halo2(m):
                """2-row top/bottom halo fill for a TROWS buffer."""
                nc.sync.dma_start(m[1:P, 0:2, :], m[0:P - 1, R:R + 2, :])
                nc.sync.dma_start(m[0:P - 1, TROWS - 2:TROWS, :], m[1:P, 2:4, :])
                for hr in (0, 1):
                    nc.sync.dma_start(m[0:1, hr:hr + 1, :], m[0:1, 2:3, :])
                for hr in (TROWS - 2, TROWS - 1):
                    nc.sync.dma_start(m[P - 1:P, hr:hr + 1, :],
                                      m[P - 1:P, TROWS - 3:TROWS - 2, :])

            def pe_sums(q, img, p_t, buf, rofs, cofs, r0=0, r1=R):
                """sum(p*m) on PE via diagonal-accumulate matmuls; sum(m) on
                PE via a ones-matmul chain through the shared msum bank,
                read out by ACT into msums1 before the next chain reuses it."""
                for r in range(r0, r1):
                    for c0 in (0, 512):
                        nc.tensor.matmul(
                            ps_ms[:, 0:512], ones16[:],
                            buf[:, rofs + r, cofs + c0:cofs + c0 + 512],
                            start=(r == 0 and c0 == 0),
                            stop=(r == R - 1 and c0 == 512))
                    for (c0, cw) in iblk:
                        nc.tensor.matmul(
                            ps_q[q][0:cw, 0:cw], p_t[:, r, c0:c0 + cw],
                            buf[:, rofs + r, cofs + c0:cofs + c0 + cw],
                            start=(diag_cnt[q] == 0),
                            stop=(diag_cnt[q] == DIAG_TOT - 1))
                        diag_cnt[q] += 1
                if r1 < R:
                    return
                nc.scalar.activation(scr512[:], ps_ms[:], COPY,
                                     accum_out=msums1[:, img * 6 + q:
                                                      img * 6 + q + 1])
                if diag_cnt[q] == DIAG_TOT:
                    # chain complete: extract the PSUM diagonal now, off the
                    # critical tail
                    nc.scalar.activation(scr[:], ps_q[q][:], COPY)
                    nc.vector.tensor_tensor(scr2[:], scr[:], ident[:], op=MULT)
                    nc.vector.tensor_reduce(sums[:, 8 + q:9 + q], scr2[:],
                                            axis=mybir.AxisListType.X, op=ADD)

            for img in range(n_img):
                t_view = t_dram[img].rearrange("(p r) w -> p r w", p=P)
                p_view = p_dram[img].rearrange("(p r) w -> p r w", p=P)
                t = img_pool.tile([P, TROWS, C], f16, tag="t", name="t")
                p_t = img_pool.tile([P, R, C], f16, tag="p", name="p")
                for r0 in range(0, R, CH):
                    st = stage_pool.tile([P, CH, C], f32, tag="stage", name="stage")
                    nc.sync.dma_start(st[:], t_view[:, r0:r0 + CH, :])
                    if img == 0 and r0 >= R // 2:
                        # first image: DVE (idle at startup) casts the second
                        # half of t so it isn't bound by the serial ACT casts
                        nc.vector.tensor_copy(t[:, 2 + r0:2 + r0 + CH, :], st[:])
                    else:
                        nc.scalar.activation(t[:, 2 + r0:2 + r0 + CH, :], st[:],
                                             COPY)
                halo2(t)
                for r0 in range(0, R, CH):
                    st = stage_pool.tile([P, CH, C], f32, tag="stage", name="stage")
                    nc.sync.dma_start(st[:], p_view[:, r0:r0 + CH, :])
                    nc.scalar.activation(p_t[:, r0:r0 + CH, :], st[:], COPY,
                                         accum_out=sums[:, img * 4 + r0 // CH:
                                                        img * 4 + r0 // CH + 1])

                for is_dil, base_q in ((True, 0), (False, 3)):
                    OP = MAX if is_dil else MIN
                    fill = -1e4 if is_dil else 1e4
                    V = nc.vector
                    first = img == 0 and is_dil
                    rsp = ((0, 2), (2, 4), (4, 6), (6, 8)) if first else ((0, 8),)

                    V.memset(m3[:, :, 0:1], fill)
                    V.memset(m3[:, :, MC - 1:MC], fill)
                    V.memset(m5[:, :, 0:1], fill)
                    V.memset(m5[:, :, MC - 1:MC], fill)

                    def emit_h1(OP=OP, rsp=rsp):
                        # h1 = hmax3(t); row-split on the first side so DVE
                        # starts as t chunks land
                        for (ra, rb) in rsp:
                            V.tensor_tensor(h1[:, ra:rb, 1:C - 1],
                                            t[:, 2 + ra:2 + rb, 0:C - 2],
                                            t[:, 2 + ra:2 + rb, 2:C], op=OP)
                        V.tensor_tensor(h1[:, :, 0:1], t[:, 2:10, 1:2],
                                        t[:, 2:10, 1:2], op=OP)
                        V.tensor_tensor(h1[:, :, C - 1:C],
                                        t[:, 2:10, C - 2:C - 1],
                                        t[:, 2:10, C - 2:C - 1], op=OP)
                        for (ra, rb) in rsp:
                            V.tensor_tensor(h1[:, ra:rb, :], h1[:, ra:rb, :],
                                            t[:, 2 + ra:2 + rb, :], op=OP)

                    def emit_hm2(OP=OP, rsp=rsp):
                        # hm2 = op(t l2, t r2); halo rows via DMA. Emitted
                        # early so the halo DMAs hide behind later DVE work.
                        for (ra, rb) in rsp:
                            V.tensor_tensor(hm2[:, 2 + ra:2 + rb, 2:C - 2],
                                            t[:, 2 + ra:2 + rb, 0:C - 4],
                                            t[:, 2 + ra:2 + rb, 4:C], op=OP)
                        V.tensor_tensor(hm2[:, 2:10, 0:2], t[:, 2:10, 2:4],
                                        t[:, 2:10, 2:4], op=OP)
                        V.tensor_tensor(hm2[:, 2:10, C - 2:C],
                                        t[:, 2:10, C - 4:C - 2],
                                        t[:, 2:10, C - 4:C - 2], op=OP)
                        halo2(hm2)

                    if first:
                        emit_h1()
                        emit_hm2()
                    else:
                        emit_hm2()
                        emit_h1()

                    # ---- m3 = op(h1, t up1, t dn1) ----
                    V.tensor_tensor(m3[:, 1:9, 1:C + 1], h1[:], t[:, 1:9, :], op=OP)
                    V.tensor_tensor(m3[:, 1:9, 1:C + 1], m3[:, 1:9, 1:C + 1],
                                    t[:, 3:11, :], op=OP)
                    halo1(m3)
                    pe_sums(base_q + 0, img, p_t, m3, 1, 1)

                    # ---- m7 init = op(hm2 up2, dn2) ----
                    V.tensor_tensor(m7[:], hm2[:, 0:8, :], hm2[:, 4:12, :], op=OP)

                    # ---- m5 = op(m3 l1, r1, up1, dn1) ----
                    V.tensor_tensor(m5[:, 1:9, 1:C + 1], m3[:, 1:9, 0:C],
                                    m3[:, 1:9, 2:C + 2], op=OP)
                    V.tensor_tensor(m5[:, 1:9, 1:C + 1], m5[:, 1:9, 1:C + 1],
                                    m3[:, 0:8, 1:C + 1], op=OP)
                    V.tensor_tensor(m5[:, 1:9, 1:C + 1], m5[:, 1:9, 1:C + 1],
                                    m3[:, 2:10, 1:C + 1], op=OP)
                    halo1(m5)
                    pe_sums(base_q + 1, img, p_t, m5, 1, 1)

                    # ---- m7 folds: op(m7, m5 l1, r1, up1, dn1) ----
                    last = img == n_img - 1 and not is_dil
                    V.tensor_tensor(m7[:], m7[:], m5[:, 1:9, 0:C], op=OP)
                    V.tensor_tensor(m7[:], m7[:], m5[:, 1:9, 2:C + 2], op=OP)
                    V.tensor_tensor(m7[:], m7[:], m5[:, 0:8, 1:C + 1], op=OP)
                    for (ra, rb) in (((0, 2), (2, 4), (4, 6), (6, 8)) if last
                                     else ((0, 8),)):
                        V.tensor_tensor(m7[:, ra:rb, :], m7[:, ra:rb, :],
                                        m5[:, 2 + ra:2 + rb, 1:C + 1], op=OP)
                        pe_sums(base_q + 2, img, p_t, m7, 0, 0, r0=ra, r1=rb)

            # ---- epilogue ----
            ps_fin = psum_pool.tile([1, NSUM], f32, tag="psfin")
            nc.tensor.matmul(ps_fin[:], ones32[:], sums[:], start=True, stop=True)
            nc.scalar.activation(outsb[:, 0:NSUM], ps_fin[:], COPY)
            nc.scalar.activation(outsb[:, NSUM:NSUM + 12], msums1[:], COPY)
            nc.sync.dma_start(out_dram[:], outsb[:])

    nc.compile()
    return nc


def combine_partials(partials):
    """partials: [ncores, 26] float32 -> scalar loss (mirrors reference)."""
    partials = np.asarray(partials, dtype=np.float64)
    p_sum = partials[:, 0:8].sum()
    total = 0.0
    for q in range(6):
        inter = partials[:, 8 + q].sum()
        msum = sum(partials[:, 14 + img * 6 + q].sum() for img in range(BPC))
        card = p_sum + msum
        score = 2.0 * inter / max(card, EPS)
        total += (1.0 - score) * (1.0 if msum > 0 else 0.0)
    return np.float32(total / 3.0)


def kernel(pred_student_prob, teacher_prob):
    from concourse.bass_utils import run_bass_kernel_spmd

    if "nc" not in _CACHE:
        _CACHE["nc"] = build_nc()
    nc = _CACHE["nc"]

    pred = np.ascontiguousarray(pred_student_prob.reshape(B, H, W), dtype=np.float32)
    teach = np.ascontiguousarray(teacher_prob.reshape(B, H, W), dtype=np.float32)
    ident = np.eye(P, dtype=np.float32)
    in_maps = []
    for c in range(NCORES):
        sl = slice(c * BPC, (c + 1) * BPC)
        in_maps.append({
            "teacher": np.ascontiguousarray(teach[sl]),
            "pred": np.ascontiguousarray(pred[sl]),
            "ident": ident,
        })
    res = run_bass_kernel_spmd(nc, in_maps, core_ids=list(range(NCORES)))
    partials = np.stack([res.results[c]["partials"][0] for c in range(NCORES)])
    return combine_partials(partials)


# revision 18
# speedup vs baseline: 1.4535x; 1.0238x over previous
"""Trainium2 Bass kernel for nn_LossConsistenciaMorfologicaCompuesta.

Composite morphological-consistency loss:
  for k in (3,5,7): Dice(pred, dilate_k(teacher)) + Dice(pred, erode_k(teacher)),
  total/3, cv2-ellipse structuring elements, Dice reduced over (batch, pixels).

Strategy (8 NeuronCores, data-parallel over batch B=16 -> 2 images/core):
  - Slab layout: one 1024x1024 image in SBUF as [128 partitions, 8 rows(+halo),
    1024 cols] fp16 (DVE 2x mode). Vertical shifts are free-dim row offsets;
    halo rows come from tiny partition-shifted SBUF->SBUF DMAs (replicate at
    image edges -- exact for flat morphology).
  - Ellipse decomposition (exact, 13 DVE folds per side):
      h1   = hmax3(t);  m3 = max(h1, t up1, t dn1)          (ellipse3 = plus)
      m5   = max(m3 l1, r1, up1, dn1)                        (ellipse5 = diamond2)
      hm2  = max(t l2, t r2)  (halo rows via DMA)
      m7   = max(hm2 up2, hm2 dn2, m5 l1, r1, up1, dn1)     (ellipse7)
    erosion mirrored with min. DVE is the only engine with elementwise
    max/min on this target, so everything else rides other engines:
  - sum(p*m) on PE: "diagonal" matmuls (weights = 128-col blocks of p,
    moving = m blocks) accumulated into a [128,128] PSUM region whose
    diagonal holds the inter partials; extracted once at the end with a
    tensor_tensor_reduce against an identity matrix.
  - sum(m) on PE: ones-weight matmuls into a [1,384] PSUM region sharing
    the same bank as the diag tile (PSUM is bank-granular, 8 banks).
  - sum(p) rides the f32->f16 cast on ACT as accum_out.
  - Each core writes 20 partial sums; the host combines into the scalar.
"""

import numpy as np

B, C_IN, H, W = 16, 1, 1024, 1024
NCORES = 8
BPC = B // NCORES      # images per core
P = 128                # SBUF partitions
R = H // P             # 8 slab rows per partition
EPS = 1e-7
CH = 2                 # slab rows per f32 staging chunk
NSUM = 18              # sums cols: 8 p-cast + 6 inter + 4 tail-msum
NOUT = 30              # 18 sums + 12 msum scalars

_CACHE = {}


def build_nc():
    import concourse.bacc as bacc
    import concourse.mybir as mybir
    import concourse.tile as tile

    f32 = mybir.dt.float32
    f16 = mybir.dt.float16
    MAX = mybir.AluOpType.max
    MIN = mybir.AluOpType.min
    ADD = mybir.AluOpType.add
    MULT = mybir.AluOpType.mult
    COPY = mybir.ActivationFunctionType.Copy

    C = W
    n_img = BPC
    TROWS = R + 4   # t/hm2: 2 halo rows above+below
    MROWS = R + 2   # m3/m5: 1 halo row above+below
    MC = C + 2      # m3/m5: 1 pad col each side (buf col = img col + 1)

    nc = bacc.Bacc("TRN2", target_bir_lowering=False)
    t_dram = nc.dram_tensor("teacher", [n_img, R * P, C], f32, kind="ExternalInput")
    p_dram = nc.dram_tensor("pred", [n_img, R * P, C], f32, kind="ExternalInput")
    i_dram = nc.dram_tensor("ident", [P, P], f32, kind="ExternalInput")
    out_dram = nc.dram_tensor("partials", [1, NOUT], f32, kind="ExternalOutput")

    with tile.TileContext(nc) as tc:
        with (
            tc.tile_pool(name="stage", bufs=2) as stage_pool,
            tc.tile_pool(name="img", bufs=2) as img_pool,
            tc.tile_pool(name="imgp", bufs=1) as imgp_pool,
            tc.tile_pool(name="morph2", bufs=3) as morph2_pool,
            tc.tile_pool(name="morph", bufs=1) as morph_pool,
            tc.tile_pool(name="small", bufs=1) as small_pool,
            tc.tile_pool(name="psum", bufs=1, space="PSUM") as psum_pool,
        ):
            sums = small_pool.tile([P, NSUM], f32, tag="sums")
            ones16 = small_pool.tile([P, 1], f16, tag="ones16")
            ones32 = small_pool.tile([P, 1], f32, tag="ones32")
            ident = small_pool.tile([P, P], f32, tag="ident")
            scr = small_pool.tile([P, P], f32, tag="scr")
            scr512 = small_pool.tile([1, 512], f32, tag="scr512")
            scr2 = small_pool.tile([P, P], f32, tag="scr2")
            msums1 = small_pool.tile([1, 12], f32, tag="msums1")
            outsb = small_pool.tile([1, NOUT], f32, tag="outsb")
            nc.vector.memset(sums[:], 0.0)
            nc.vector.memset(ones16[:], 1.0)
            nc.vector.memset(ones32[:], 1.0)
            nc.sync.dma_start(ident[:], i_dram[:])

            m5 = morph_pool.tile([P, MROWS, MC], f16, tag="m5")
            hm2 = morph_pool.tile([P, TROWS, C], f16, tag="hm2")
            m7 = morph_pool.tile([P, R, C], f16, tag="m7")

            # one PSUM bank per quantity holding the [128,128] diag tile,
            # plus a single shared [1,512] bank for the msum chains
            ps_q = [psum_pool.tile([P, P], f32, tag=f"psq{q}", name=f"psq{q}")
                    for q in range(6)]
            ps_ms = [psum_pool.tile([1, 512], f32, tag="psms0", name="psms0"),
                     psum_pool.tile([1, 512], f32, tag="psms1", name="psms1")]
            diag_cnt = [0] * 6
            iblk = [(c0, 128) for c0 in range(0, C, 128)]
            DIAG_TOT = n_img * R * len(iblk)

            def halo1(m):
                """1-row top/bottom halo fill (replicate at image edges)."""
                nc.sync.dma_start(m[1:P, 0:1, :], m[0:P - 1, R:R + 1, :])
                nc.sync.dma_start(m[0:P - 1, MROWS - 1:MROWS, :], m[1:P, 1:2, :])
                nc.sync.dma_start(m[0:1, 0:1, :], m[0:1, 1:2, :])
                nc.sync.dma_start(m[P - 1:P, MROWS - 1:MROWS, :],
                                  m[P - 1:P, MROWS - 2:MROWS - 1, :])

            def halo2(m):
                """2-row top/bottom halo fill for a TROWS buffer."""
                nc.sync.dma_start(m[1:P, 0:2, :], m[0:P - 1, R:R + 2, :])
                nc.sync.dma_start(m[0:P - 1, TROWS - 2:TROWS, :], m[1:P, 2:4, :])
                for hr in (0, 1):
                    nc.sync.dma_start(m[0:1, hr:hr + 1, :], m[0:1, 2:3, :])
                for hr in (TROWS - 2, TROWS - 1):
                    nc.sync.dma_start(m[P - 1:P, hr:hr + 1, :],
                                      m[P - 1:P, TROWS - 3:TROWS - 2, :])

            def pe_sums(q, img, p_t, buf, rofs, cofs, r0=0, r1=R,
                        msum_pe=True):
                """sum(p*m) on PE via diagonal-accumulate matmuls; sum(m) on
                PE via a ones-matmul chain through the shared msum bank,
                read out by ACT into msums1 before the next chain reuses it."""
                for r in range(r0, r1):
                    if msum_pe:
                        for c0 in (0, 512):
                            nc.tensor.matmul(
                                ps_ms[q % 2][:, 0:512], ones16[:],
                                buf[:, rofs + r, cofs + c0:cofs + c0 + 512],
                                start=(r == 0 and c0 == 0),
                                stop=(r == R - 1 and c0 == 512))
                    for (c0, cw) in iblk:
                        nc.tensor.matmul(
                            ps_q[q][0:cw, 0:cw], p_t[:, r, c0:c0 + cw],
                            buf[:, rofs + r, cofs + c0:cofs + c0 + cw],
                            start=(diag_cnt[q] == 0),
                            stop=(diag_cnt[q] == DIAG_TOT - 1))
                        diag_cnt[q] += 1
                if r1 < R:
                    return
                if msum_pe:
                    nc.scalar.activation(scr512[:], ps_ms[q % 2][:], COPY,
                                         accum_out=msums1[:, img * 6 + q:
                                                          img * 6 + q + 1])
                if diag_cnt[q] == DIAG_TOT:
                    # chain complete: extract the PSUM diagonal now, off the
                    # critical tail
                    nc.scalar.activation(scr[:], ps_q[q][:], COPY)
                    nc.vector.tensor_tensor(scr2[:], scr[:], ident[:], op=MULT)
                    nc.vector.tensor_reduce(sums[:, 8 + q:9 + q], scr2[:],
                                            axis=mybir.AxisListType.X, op=ADD)

            for img in range(n_img):
                t_view = t_dram[img].rearrange("(p r) w -> p r w", p=P)
                p_view = p_dram[img].rearrange("(p r) w -> p r w", p=P)
                t = img_pool.tile([P, TROWS, C], f16, tag="t", name="t")
                p_t = imgp_pool.tile([P, R, C], f16, tag="p", name="p")
                for (ra, rb) in ((0, 2), (2, 4), (4, 6), (6, 8)):
                    st = stage_pool.tile([P, CH, C], f32, tag="stage", name="stage")
                    nc.sync.dma_start(st[:], t_view[:, ra:rb, :])
                    if img == 0 and ra >= R // 2:
                        # first image: DVE (idle at startup) casts the second
                        # half of t so it isn't bound by the serial ACT casts
                        nc.vector.tensor_copy(t[:, 2 + ra:2 + rb, :], st[:])
                    else:
                        nc.scalar.activation(t[:, 2 + ra:2 + rb, :], st[:],
                                             COPY)
                halo2(t)
                for r0 in range(0, R, CH):
                    st = stage_pool.tile([P, CH, C], f32, tag="stage", name="stage")
                    nc.sync.dma_start(st[:], p_view[:, r0:r0 + CH, :])
                    nc.scalar.activation(p_t[:, r0:r0 + CH, :], st[:], COPY,
                                         accum_out=sums[:, img * 4 + r0 // CH:
                                                        img * 4 + r0 // CH + 1])

                for is_dil, base_q in ((True, 0), (False, 3)):
                    OP = MAX if is_dil else MIN
                    fill = -1e4 if is_dil else 1e4
                    V = nc.vector
                    m3 = morph2_pool.tile([P, MROWS, MC], f16, tag="m3",
                                          name="m3")
                    first = img == 0 and is_dil
                    rsp = (((0, 2), (2, 4), (4, 6), (6, 8)) if first
                           else ((0, 8),))

                    V.memset(m3[:, :, 0:1], fill)
                    V.memset(m3[:, :, MC - 1:MC], fill)
                    V.memset(m5[:, :, 0:1], fill)
                    V.memset(m5[:, :, MC - 1:MC], fill)

                    def emit_h1(OP=OP, rsp=rsp):
                        # hmax3(t) built directly into m3 (no h1 buffer):
                        # m3 = op(t l1, t r1); then op= t center
                        for (ra, rb) in rsp:
                            V.tensor_tensor(m3[:, 1 + ra:1 + rb, 2:C],
                                            t[:, 2 + ra:2 + rb, 0:C - 2],
                                            t[:, 2 + ra:2 + rb, 2:C], op=OP)
                        V.tensor_tensor(m3[:, 1:9, 1:2], t[:, 2:10, 1:2],
                                        t[:, 2:10, 1:2], op=OP)
                        V.tensor_tensor(m3[:, 1:9, C:C + 1],
                                        t[:, 2:10, C - 2:C - 1],
                                        t[:, 2:10, C - 2:C - 1], op=OP)
                        for (ra, rb) in rsp:
                            V.tensor_tensor(m3[:, 1 + ra:1 + rb, 1:C + 1],
                                            m3[:, 1 + ra:1 + rb, 1:C + 1],
                                            t[:, 2 + ra:2 + rb, :], op=OP)

                    def emit_hm2(OP=OP, rsp=rsp):
                        # hm2 = op(t l2, t r2); halo rows via DMA. Emitted
                        # early so the halo DMAs hide behind later DVE work.
                        for (ra, rb) in rsp:
                            V.tensor_tensor(hm2[:, 2 + ra:2 + rb, 2:C - 2],
                                            t[:, 2 + ra:2 + rb, 0:C - 4],
                                            t[:, 2 + ra:2 + rb, 4:C], op=OP)
                        V.tensor_tensor(hm2[:, 2:10, 0:2], t[:, 2:10, 2:4],
                                        t[:, 2:10, 2:4], op=OP)
                        V.tensor_tensor(hm2[:, 2:10, C - 2:C],
                                        t[:, 2:10, C - 4:C - 2],
                                        t[:, 2:10, C - 4:C - 2], op=OP)
                        halo2(hm2)

                    if first:
                        emit_h1()
                        emit_hm2()
                    else:
                        emit_hm2()
                        emit_h1()

                    # ---- m3 = op(hmax3, t up1, t dn1) ----
                    V.tensor_tensor(m3[:, 1:9, 1:C + 1], m3[:, 1:9, 1:C + 1],
                                    t[:, 1:9, :], op=OP)
                    V.tensor_tensor(m3[:, 1:9, 1:C + 1], m3[:, 1:9, 1:C + 1],
                                    t[:, 3:11, :], op=OP)
                    halo1(m3)
                    pe_sums(base_q + 0, img, p_t, m3, 1, 1)

                    # ---- m7 init = op(hm2 up2, dn2) ----
                    V.tensor_tensor(m7[:], hm2[:, 0:8, :], hm2[:, 4:12, :], op=OP)

                    # ---- m5 = op(m3 l1, r1, up1, dn1) ----
                    V.tensor_tensor(m5[:, 1:9, 1:C + 1], m3[:, 1:9, 0:C],
                                    m3[:, 1:9, 2:C + 2], op=OP)
                    V.tensor_tensor(m5[:, 1:9, 1:C + 1], m5[:, 1:9, 1:C + 1],
                                    m3[:, 0:8, 1:C + 1], op=OP)
                    V.tensor_tensor(m5[:, 1:9, 1:C + 1], m5[:, 1:9, 1:C + 1],
                                    m3[:, 2:10, 1:C + 1], op=OP)
                    halo1(m5)
                    pe_sums(base_q + 1, img, p_t, m5, 1, 1)

                    # ---- m7 folds: op(m7, m5 l1, r1, up1, dn1) ----
                    # Last side: fully row-quartered so PE/ACT consumption
                    # pipelines with the folds and the tail shrinks.
                    last = img == n_img - 1 and not is_dil
                    if last:
                        for qi, (ra, rb) in enumerate(
                                ((0, 2), (2, 4), (4, 6), (6, 8))):
                            V.tensor_tensor(m7[:, ra:rb, :], m7[:, ra:rb, :],
                                            m5[:, 1 + ra:1 + rb, 0:C], op=OP)
                            V.tensor_tensor(m7[:, ra:rb, :], m7[:, ra:rb, :],
                                            m5[:, 1 + ra:1 + rb, 2:C + 2],
                                            op=OP)
                            V.tensor_tensor(m7[:, ra:rb, :], m7[:, ra:rb, :],
                                            m5[:, ra:rb, 1:C + 1], op=OP)
                            V.tensor_tensor(m7[:, ra:rb, :], m7[:, ra:rb, :],
                                            m5[:, 2 + ra:2 + rb, 1:C + 1],
                                            op=OP)
                            pe_sums(base_q + 2, img, p_t, m7, 0, 0,
                                    r0=ra, r1=rb, msum_pe=False)
                            nc.scalar.activation(
                                m7[:, ra:rb, :], m7[:, ra:rb, :], COPY,
                                accum_out=sums[:, 14 + qi:15 + qi])
                    else:
                        V.tensor_tensor(m7[:], m7[:], m5[:, 1:9, 0:C], op=OP)
                        V.tensor_tensor(m7[:], m7[:], m5[:, 1:9, 2:C + 2], op=OP)
                        V.tensor_tensor(m7[:], m7[:], m5[:, 0:8, 1:C + 1], op=OP)
                        V.tensor_tensor(m7[:], m7[:], m5[:, 2:10, 1:C + 1], op=OP)
                        pe_sums(base_q + 2, img, p_t, m7, 0, 0)

            # ---- epilogue ----
            nc.tensor.matmul(ps_ms[0][0:1, 0:NSUM], ones32[:], sums[:],
                             start=True, stop=True)
            nc.scalar.activation(outsb[:, 0:NSUM], ps_ms[0][0:1, 0:NSUM], COPY)
            nc.scalar.activation(outsb[:, NSUM:NSUM + 12], msums1[:], COPY)
            nc.sync.dma_start(out_dram[:], outsb[:])

    nc.compile()
    return nc


def combine_partials(partials):
    """partials: [ncores, 26] float32 -> scalar loss (mirrors reference)."""
    partials = np.asarray(partials, dtype=np.float64)
    p_sum = partials[:, 0:8].sum()
    total = 0.0
    for q in range(6):
        inter = partials[:, 8 + q].sum()
        msum = 0.0
        for img in range(BPC):
            if img == BPC - 1 and q == 5:
                msum += partials[:, 14:18].sum()
            else:
                msum += partials[:, 18 + img * 6 + q].sum()
        card = p_sum + msum
        score = 2.0 * inter / max(card, EPS)
        total += (1.0 - score) * (1.0 if msum > 0 else 0.0)
    return np.float32(total / 3.0)


def kernel(pred_student_prob, teacher_prob):
    from concourse.bass_utils import run_bass_kernel_spmd

    if "nc" not in _CACHE:
        _CACHE["nc"] = build_nc()
    nc = _CACHE["nc"]

    pred = np.ascontiguousarray(pred_student_prob.reshape(B, H, W), dtype=np.float32)
    teach = np.ascontiguousarray(teacher_prob.reshape(B, H, W), dtype=np.float32)
    ident = np.eye(P, dtype=np.float32)
    in_maps = []
    for c in range(NCORES):
        sl = slice(c * BPC, (c + 1) * BPC)
        in_maps.append({
            "teacher": np.ascontiguousarray(teach[sl]),
            "pred": np.ascontiguousarray(pred[sl]),
            "ident": ident,
        })
    res = run_bass_kernel_spmd(nc, in_maps, core_ids=list(range(NCORES)))
    partials = np.stack([res.results[c]["partials"][0] for c in range(NCORES)])
    return combine_partials(partials)


# revision 21
# speedup vs baseline: 1.4855x; 1.0220x over previous
"""Trainium2 Bass kernel for nn_LossConsistenciaMorfologicaCompuesta.

Composite morphological-consistency loss:
  for k in (3,5,7): Dice(pred, dilate_k(teacher)) + Dice(pred, erode_k(teacher)),
  total/3, cv2-ellipse structuring elements, Dice reduced over (batch, pixels).

Strategy (8 NeuronCores, data-parallel over batch B=16 -> 2 images/core):
  - Slab layout: one 1024x1024 image in SBUF as [128 partitions, 8 rows(+halo),
    1024 cols] fp16 (DVE 2x mode). Vertical shifts are free-dim row offsets;
    halo rows come from tiny partition-shifted SBUF->SBUF DMAs (replicate at
    image edges -- exact for flat morphology).
  - Ellipse decomposition (exact, 13 DVE folds per side):
      h1   = hmax3(t);  m3 = max(h1, t up1, t dn1)          (ellipse3 = plus)
      m5   = max(m3 l1, r1, up1, dn1)                        (ellipse5 = diamond2)
      hm2  = max(t l2, t r2)  (halo rows via DMA)
      m7   = max(hm2 up2, hm2 dn2, m5 l1, r1, up1, dn1)     (ellipse7)
    erosion mirrored with min. DVE is the only engine with elementwise
    max/min on this target, so everything else rides other engines:
  - sum(p*m) on PE: "diagonal" matmuls (weights = 128-col blocks of p,
    moving = m blocks) accumulated into a [128,128] PSUM region whose
    diagonal holds the inter partials; extracted once at the end with a
    tensor_tensor_reduce against an identity matrix.
  - sum(m) on PE: ones-weight matmuls into a [1,384] PSUM region sharing
    the same bank as the diag tile (PSUM is bank-granular, 8 banks).
  - sum(p) rides the f32->f16 cast on ACT as accum_out.
  - Each core writes 20 partial sums; the host combines into the scalar.
"""

import numpy as np

B, C_IN, H, W = 16, 1, 1024, 1024
NCORES = 8
BPC = B // NCORES      # images per core
P = 128                # SBUF partitions
R = H // P             # 8 slab rows per partition
EPS = 1e-7
CH = 2                 # slab rows per f32 staging chunk
NSUM = 18              # sums cols: 8 p-cast + 6 inter + 4 tail-msum
NOUT = 30              # 18 sums + 12 msum scalars

_CACHE = {}


def build_nc():
    import concourse.bacc as bacc
    import concourse.mybir as mybir
    import concourse.tile as tile

    f32 = mybir.dt.float32
    f16 = mybir.dt.float16
    MAX = mybir.AluOpType.max
    MIN = mybir.AluOpType.min
    ADD = mybir.AluOpType.add
    MULT = mybir.AluOpType.mult
    COPY = mybir.ActivationFunctionType.Copy

    C = W
    n_img = BPC
    TROWS = R + 4   # t/hm2: 2 halo rows above+below
    MROWS = R + 2   # m3/m5: 1 halo row above+below
    MC = C + 2      # m3/m5: 1 pad col each side (buf col = img col + 1)

    nc = bacc.Bacc("TRN2", target_bir_lowering=False)
    t_dram = nc.dram_tensor("teacher", [n_img, R * P, C], f32, kind="ExternalInput")
    p_dram = nc.dram_tensor("pred", [n_img, R * P, C], f32, kind="ExternalInput")
    i_dram = nc.dram_tensor("ident", [P, P], f32, kind="ExternalInput")
    out_dram = nc.dram_tensor("partials", [1, NOUT], f32, kind="ExternalOutput")

    with tile.TileContext(nc) as tc:
        with (
            tc.tile_pool(name="stage", bufs=2) as stage_pool,
            tc.tile_pool(name="img", bufs=2) as img_pool,
            tc.tile_pool(name="imgp", bufs=1) as imgp_pool,
            tc.tile_pool(name="morph2", bufs=2) as morph2_pool,
            tc.tile_pool(name="morph", bufs=1) as morph_pool,
            tc.tile_pool(name="small", bufs=1) as small_pool,
            tc.tile_pool(name="psum", bufs=1, space="PSUM") as psum_pool,
        ):
            sums = small_pool.tile([P, NSUM], f32, tag="sums")
            ones16 = small_pool.tile([P, 1], f16, tag="ones16")
            ones32 = small_pool.tile([P, 1], f32, tag="ones32")
            ident = small_pool.tile([P, P], f32, tag="ident")
            scr = small_pool.tile([P, P], f32, tag="scr")
            scr512 = small_pool.tile([1, 512], f32, tag="scr512")
            scr2 = small_pool.tile([P, P], f32, tag="scr2")
            msums1 = small_pool.tile([1, 12], f32, tag="msums1")
            outsb = small_pool.tile([1, NOUT], f32, tag="outsb")
            nc.vector.memset(sums[:], 0.0)
            nc.vector.memset(ones16[:], 1.0)
            nc.vector.memset(ones32[:], 1.0)
            nc.sync.dma_start(ident[:], i_dram[:])

            m5 = morph_pool.tile([P, MROWS, MC], f16, tag="m5")
            hm2 = morph_pool.tile([P, TROWS, C], f16, tag="hm2")

            # one PSUM bank per quantity holding the [128,128] diag tile,
            # plus a single shared [1,512] bank for the msum chains
            ps_q = [psum_pool.tile([P, P], f32, tag=f"psq{q}", name=f"psq{q}")
                    for q in range(6)]
            ps_ms = [psum_pool.tile([1, 512], f32, tag="psms0", name="psms0"),
                     psum_pool.tile([1, 512], f32, tag="psms1", name="psms1")]
            diag_cnt = [0] * 6
            iblk = [(c0, 128) for c0 in range(0, C, 128)]
            DIAG_TOT = n_img * R * len(iblk)

            def halo1(m):
                """1-row top/bottom halo fill (replicate at image edges)."""
                nc.sync.dma_start(m[1:P, 0:1, :], m[0:P - 1, R:R + 1, :])
                nc.sync.dma_start(m[0:P - 1, MROWS - 1:MROWS, :], m[1:P, 1:2, :])
                nc.sync.dma_start(m[0:1, 0:1, :], m[0:1, 1:2, :])
                nc.sync.dma_start(m[P - 1:P, MROWS - 1:MROWS, :],
                                  m[P - 1:P, MROWS - 2:MROWS - 1, :])

            def halo2(m):
                """2-row top/bottom halo fill for a TROWS buffer."""
                nc.sync.dma_start(m[1:P, 0:2, :], m[0:P - 1, R:R + 2, :])
                nc.sync.dma_start(m[0:P - 1, TROWS - 2:TROWS, :], m[1:P, 2:4, :])
                for hr in (0, 1):
                    nc.sync.dma_start(m[0:1, hr:hr + 1, :], m[0:1, 2:3, :])
                for hr in (TROWS - 2, TROWS - 1):
                    nc.sync.dma_start(m[P - 1:P, hr:hr + 1, :],
                                      m[P - 1:P, TROWS - 3:TROWS - 2, :])

            def pe_sums(q, img, p_t, buf, rofs, cofs, r0=0, r1=R,
                        msum_pe=True):
                """sum(p*m) on PE via diagonal-accumulate matmuls; sum(m) on
                PE via a ones-matmul chain through the shared msum bank,
                read out by ACT into msums1 before the next chain reuses it."""
                for r in range(r0, r1):
                    if msum_pe:
                        for c0 in (0, 512):
                            nc.tensor.matmul(
                                ps_ms[q % 2][:, 0:512], ones16[:],
                                buf[:, rofs + r, cofs + c0:cofs + c0 + 512],
                                start=(r == 0 and c0 == 0),
                                stop=(r == R - 1 and c0 == 512))
                    for (c0, cw) in iblk:
                        nc.tensor.matmul(
                            ps_q[q][0:cw, 0:cw], p_t[:, r, c0:c0 + cw],
                            buf[:, rofs + r, cofs + c0:cofs + c0 + cw],
                            start=(diag_cnt[q] == 0),
                            stop=(diag_cnt[q] == DIAG_TOT - 1))
                        diag_cnt[q] += 1
                if r1 < R:
                    return
                if msum_pe:
                    nc.scalar.activation(scr512[:], ps_ms[q % 2][:], COPY,
                                         accum_out=msums1[:, img * 6 + q:
                                                          img * 6 + q + 1])
                if diag_cnt[q] == DIAG_TOT:
                    # chain complete: extract the PSUM diagonal now, off the
                    # critical tail
                    nc.scalar.activation(scr[:], ps_q[q][:], COPY)
                    nc.vector.tensor_tensor(scr2[:], scr[:], ident[:], op=MULT)
                    nc.vector.tensor_reduce(sums[:, 8 + q:9 + q], scr2[:],
                                            axis=mybir.AxisListType.X, op=ADD)

            for img in range(n_img):
                t_view = t_dram[img].rearrange("(p r) w -> p r w", p=P)
                p_view = p_dram[img].rearrange("(p r) w -> p r w", p=P)
                t = img_pool.tile([P, TROWS, C], f16, tag="t", name="t")
                p_t = imgp_pool.tile([P, R, C], f16, tag="p", name="p")
                for (ra, rb) in ((0, 2), (2, 4), (4, 6), (6, 8)):
                    st = stage_pool.tile([P, CH, C], f32, tag="stage", name="stage")
                    nc.sync.dma_start(st[:], t_view[:, ra:rb, :])
                    if img == 0 and ra >= R // 2:
                        # first image: DVE (idle at startup) casts the second
                        # half of t so it isn't bound by the serial ACT casts
                        nc.vector.tensor_copy(t[:, 2 + ra:2 + rb, :], st[:])
                    else:
                        nc.scalar.activation(t[:, 2 + ra:2 + rb, :], st[:],
                                             COPY)
                halo2(t)
                for r0 in range(0, R, CH):
                    st = stage_pool.tile([P, CH, C], f32, tag="stage", name="stage")
                    nc.sync.dma_start(st[:], p_view[:, r0:r0 + CH, :])
                    nc.scalar.activation(p_t[:, r0:r0 + CH, :], st[:], COPY,
                                         accum_out=sums[:, img * 4 + r0 // CH:
                                                        img * 4 + r0 // CH + 1])

                for is_dil, base_q in ((True, 0), (False, 3)):
                    OP = MAX if is_dil else MIN
                    fill = -1e4 if is_dil else 1e4
                    V = nc.vector
                    m3 = morph2_pool.tile([P, MROWS, MC], f16, tag="m3",
                                          name="m3")
                    m7 = morph2_pool.tile([P, R, C], f16, tag="m7", name="m7")
                    first = img == 0 and is_dil
                    rsp = (((0, 2), (2, 4), (4, 6), (6, 8)) if first
                           else ((0, 8),))

                    V.memset(m3[:, :, 0:1], fill)
                    V.memset(m3[:, :, MC - 1:MC], fill)
                    V.memset(m5[:, :, 0:1], fill)
                    V.memset(m5[:, :, MC - 1:MC], fill)

                    def emit_h1(OP=OP, rsp=rsp):
                        # hmax3(t) built directly into m3 (no h1 buffer):
                        # m3 = op(t l1, t r1); then op= t center
                        for (ra, rb) in rsp:
                            V.tensor_tensor(m3[:, 1 + ra:1 + rb, 2:C],
                                            t[:, 2 + ra:2 + rb, 0:C - 2],
                                            t[:, 2 + ra:2 + rb, 2:C], op=OP)
                        V.tensor_tensor(m3[:, 1:9, 1:2], t[:, 2:10, 1:2],
                                        t[:, 2:10, 1:2], op=OP)
                        V.tensor_tensor(m3[:, 1:9, C:C + 1],
                                        t[:, 2:10, C - 2:C - 1],
                                        t[:, 2:10, C - 2:C - 1], op=OP)
                        for (ra, rb) in rsp:
                            V.tensor_tensor(m3[:, 1 + ra:1 + rb, 1:C + 1],
                                            m3[:, 1 + ra:1 + rb, 1:C + 1],
                                            t[:, 2 + ra:2 + rb, :], op=OP)

                    def emit_hm2(OP=OP, rsp=rsp):
                        # hm2 = op(t l2, t r2); halo rows via DMA. Emitted
                        # early so the halo DMAs hide behind later DVE work.
                        for (ra, rb) in rsp:
                            V.tensor_tensor(hm2[:, 2 + ra:2 + rb, 2:C - 2],
                                            t[:, 2 + ra:2 + rb, 0:C - 4],
                                            t[:, 2 + ra:2 + rb, 4:C], op=OP)
                        V.tensor_tensor(hm2[:, 2:10, 0:2], t[:, 2:10, 2:4],
                                        t[:, 2:10, 2:4], op=OP)
                        V.tensor_tensor(hm2[:, 2:10, C - 2:C],
                                        t[:, 2:10, C - 4:C - 2],
                                        t[:, 2:10, C - 4:C - 2], op=OP)
                        halo2(hm2)

                    if first:
                        emit_h1()
                        emit_hm2()
                    else:
                        emit_hm2()
                        emit_h1()

                    # ---- m3 = op(hmax3, t up1, t dn1) ----
                    V.tensor_tensor(m3[:, 1:9, 1:C + 1], m3[:, 1:9, 1:C + 1],
                                    t[:, 1:9, :], op=OP)
                    V.tensor_tensor(m3[:, 1:9, 1:C + 1], m3[:, 1:9, 1:C + 1],
                                    t[:, 3:11, :], op=OP)
                    halo1(m3)
                    pe_sums(base_q + 0, img, p_t, m3, 1, 1)

                    # ---- m7 init = op(hm2 up2, dn2) ----
                    V.tensor_tensor(m7[:], hm2[:, 0:8, :], hm2[:, 4:12, :], op=OP)

                    # ---- m5 = op(m3 l1, r1, up1, dn1) ----
                    V.tensor_tensor(m5[:, 1:9, 1:C + 1], m3[:, 1:9, 0:C],
                                    m3[:, 1:9, 2:C + 2], op=OP)
                    V.tensor_tensor(m5[:, 1:9, 1:C + 1], m5[:, 1:9, 1:C + 1],
                                    m3[:, 0:8, 1:C + 1], op=OP)
                    V.tensor_tensor(m5[:, 1:9, 1:C + 1], m5[:, 1:9, 1:C + 1],
                                    m3[:, 2:10, 1:C + 1], op=OP)
                    halo1(m5)
                    pe_sums(base_q + 1, img, p_t, m5, 1, 1)

                    # ---- m7 folds: op(m7, m5 l1, r1, up1, dn1) ----
                    # Last side: fully row-quartered so PE/ACT consumption
                    # pipelines with the folds and the tail shrinks.
                    last = img == n_img - 1 and not is_dil
                    if last:
                        for qi, (ra, rb) in enumerate(
                                ((0, 2), (2, 4), (4, 6), (6, 8))):
                            V.tensor_tensor(m7[:, ra:rb, :], m7[:, ra:rb, :],
                                            m5[:, 1 + ra:1 + rb, 0:C], op=OP)
                            V.tensor_tensor(m7[:, ra:rb, :], m7[:, ra:rb, :],
                                            m5[:, 1 + ra:1 + rb, 2:C + 2],
                                            op=OP)
                            V.tensor_tensor(m7[:, ra:rb, :], m7[:, ra:rb, :],
                                            m5[:, ra:rb, 1:C + 1], op=OP)
                            V.tensor_tensor(m7[:, ra:rb, :], m7[:, ra:rb, :],
                                            m5[:, 2 + ra:2 + rb, 1:C + 1],
                                            op=OP)
                            pe_sums(base_q + 2, img, p_t, m7, 0, 0,
                                    r0=ra, r1=rb, msum_pe=False)
                            nc.scalar.activation(
                                m7[:, ra:rb, :], m7[:, ra:rb, :], COPY,
                                accum_out=sums[:, 14 + qi:15 + qi])
                    else:
                        V.tensor_tensor(m7[:], m7[:], m5[:, 1:9, 0:C], op=OP)
                        V.tensor_tensor(m7[:], m7[:], m5[:, 1:9, 2:C + 2], op=OP)
                        V.tensor_tensor(m7[:], m7[:], m5[:, 0:8, 1:C + 1], op=OP)
                        V.tensor_tensor(m7[:], m7[:], m5[:, 2:10, 1:C + 1], op=OP)
                        pe_sums(base_q + 2, img, p_t, m7, 0, 0)

            # ---- epilogue ----
            nc.tensor.matmul(ps_ms[0][0:1, 0:NSUM], ones32[:], sums[:],
                             start=True, stop=True)
            nc.scalar.activation(outsb[:, 0:NSUM], ps_ms[0][0:1, 0:NSUM], COPY)
            nc.scalar.activation(outsb[:, NSUM:NSUM + 12], msums1[:], COPY)
            nc.sync.dma_start(out_dram[:], outsb[:])

    nc.compile()
    return nc


def combine_partials(partials):
    """partials: [ncores, 26] float32 -> scalar loss (mirrors reference)."""
    partials = np.asarray(partials, dtype=np.float64)
    p_sum = partials[:, 0:8].sum()
    total = 0.0
    for q in range(6):
        inter = partials[:, 8 + q].sum()
        msum = 0.0
        for img in range(BPC):
            if img == BPC - 1 and q == 5:
                msum += partials[:, 14:18].sum()
            else:
                msum += partials[:, 18 + img * 6 + q].sum()
        card = p_sum + msum
        score = 2.0 * inter / max(card, EPS)
        total += (1.0 - score) * (1.0 if msum > 0 else 0.0)
    return np.float32(total / 3.0)


def kernel(pred_student_prob, teacher_prob):
    from concourse.bass_utils import run_bass_kernel_spmd

    if "nc" not in _CACHE:
        _CACHE["nc"] = build_nc()
    nc = _CACHE["nc"]

    pred = np.ascontiguousarray(pred_student_prob.reshape(B, H, W), dtype=np.float32)
    teach = np.ascontiguousarray(teacher_prob.reshape(B, H, W), dtype=np.float32)
    ident = np.eye(P, dtype=np.float32)
    in_maps = []
    for c in range(NCORES):
        sl = slice(c * BPC, (c + 1) * BPC)
        in_maps.append({
            "teacher": np.ascontiguousarray(teach[sl]),
            "pred": np.ascontiguousarray(pred[sl]),
            "ident": ident,
        })
    res = run_bass_kernel_spmd(nc, in_maps, core_ids=list(range(NCORES)))
    partials = np.stack([res.results[c]["partials"][0] for c in range(NCORES)])
    return combine_partials(partials)
